# revision 1
# baseline (speedup 1.0000x reference)
"""Single-head cross-attention on 8 NeuronCores, data-parallel over batch.

Math per core (batch element b):
    q = x @ Wq + bq;  k = enc @ Wk + bk;  v = enc @ Wv + bv
    out = softmax(q k^T / sqrt(H)) @ v @ Wp + bp

Layout strategy (no on-chip transposes anywhere):
    host:    xT[E,T], encT[E,S] (pre-transposed), Wq' = Wq/sqrt(H)
    qT[h,t] = Wq'^T-tiles as lhsT, xT as rhs          (+bq' per-partition)
    kT[h,s] = Wk-tiles  as lhsT, encT as rhs          (+bk  per-partition)
    v[s,h]  = encT-tiles as lhsT, Wv as rhs
    ST[s,t] = kT-tiles  as lhsT, qT as rhs            (scores, transposed)
    Ex      = exp(ST)          (no max subtraction; scores are O(1) here,
                                softmax is shift-invariant so result matches)
    r[t]    = ones^T column matmuls over Ex s-tiles   ([t,1] per t-tile)
    OT[h,t] = v-tiles   as lhsT, Ex as rhs            (unnormalized)
    Y[t,e]  = OT-tiles  as lhsT, Wp as rhs, scaled by 1/r[t] on copy-out
    bv/bp are folded into a host-side rank-1 add: softmax rows sum to 1, so
    P@(v + 1 bv^T) @ Wp + bp = P@v@Wp + 1 (bv@Wp + bp)^T exactly.

Matmuls run in float32r (fp32 bits, fast PE mode, 1 cycle/row at N>=256):
operand DRAM params and SBUF tiles are declared float32r so the BIR
verifier's rounded-producer rule is satisfied; the tiny row-sum matmuls
use plain fp32 (N=1 is illegal for fp32r). Built with bacc.Bacc so
finalize() runs the wait-splitting passes walrus codegen requires.
Measured on trn2: 149.2 us NEFF exec, rel err 3.3e-4 vs the reference.
"""

import os

import numpy as np

import concourse.bass as bass
import concourse.bacc as bacc
import concourse.tile as tile
from concourse import mybir
from concourse.bass_utils import run_bass_kernel_spmd

P = 128
B, T, S, E, H = 8, 1024, 1024, 768, 768
NE, NH, NT, NS = E // P, H // P, T // P, S // P
F32 = mybir.dt.float32
MM_DT = mybir.dt.float32r  # PE fast mode for 4-byte floats
AFT = mybir.ActivationFunctionType

_NC_CACHE = {}
LAST_RESULT = None


def _build_bass():
    nc = bacc.Bacc()
    xT_d = nc.declare_dram_parameter("xT", [E, T], MM_DT, isOutput=False)
    encT_d = nc.declare_dram_parameter("encT", [E, S], MM_DT, isOutput=False)
    wq_d = nc.declare_dram_parameter("wq", [E, H], MM_DT, isOutput=False)
    wk_d = nc.declare_dram_parameter("wk", [E, H], MM_DT, isOutput=False)
    wv_d = nc.declare_dram_parameter("wv", [E, H], MM_DT, isOutput=False)
    wp_d = nc.declare_dram_parameter("wp", [H, E], MM_DT, isOutput=False)
    bqk_d = nc.declare_dram_parameter("bqk", [2, H], F32, isOutput=False)
    out_d = nc.declare_dram_parameter("out", [T, E], F32, isOutput=True)
    rrow_d = nc.dram_tensor("rrow_bounce", [1, T], F32)

    def mm(ps, lhsT, rhs, start, stop):
        nc.tensor.matmul(ps, lhsT, rhs, start=start, stop=stop)

    with tile.TileContext(nc) as tc:
        with (
            tc.tile_pool(name="const", bufs=1) as constp,
            tc.tile_pool(name="big", bufs=1) as bigp,
            tc.tile_pool(name="psum", bufs=3, space="PSUM") as psp,
            tc.tile_pool(name="psum_r", bufs=1, space="PSUM") as psr,
            tc.tile_pool(name="yout", bufs=3) as youtp,
        ):
            bq_sb = constp.tile([P, NH], F32, tag="bq")
            bk_sb = constp.tile([P, NH], F32, tag="bk")
            ones_stg = constp.tile([P, 2], F32, tag="ones_stg")
            nc.vector.memset(ones_stg[:], 1.0)
            ones_sb = constp.tile([P, 2], MM_DT, tag="ones")
            nc.vector.tensor_copy(ones_sb[:], ones_stg[:])
            rcp_sb = constp.tile([P, NT], F32, tag="rcp")
            rrow_sb = constp.tile([P, T], F32, tag="rrow")

            # long-lived activations
            qT_sb = bigp.tile([P, NH * T], MM_DT, tag="qT")
            kT_sb = bigp.tile([P, NH * S], MM_DT, tag="kT")
            v_sb = bigp.tile([P, NS * H], MM_DT, tag="v")

            with tc.tile_pool(name="ph1", bufs=1) as ph1:
                wq_sb = ph1.tile([P, NE * H], MM_DT, tag="wq")
                xT_sb = ph1.tile([P, NE * T], MM_DT, tag="xT")
                wk_sb = ph1.tile([P, NE * H], MM_DT, tag="wk")
                encT_sb = ph1.tile([P, NE * S], MM_DT, tag="encT")
                wv_sb = ph1.tile([P, NE * H], MM_DT, tag="wv")
                def view_of(dram, j):
                    return dram[:].rearrange("(j p) t -> j p t", p=P)[j]

                # one trigger per e-tile: DMA triggers issue serially at
                # ~640ns each on the Sync engine, so fewer+bigger wins; the
                # transfers themselves are packet-spread across 16 engines
                for j in range(NE):
                    nc.sync.dma_start(wq_sb[:, j * H:(j + 1) * H], view_of(wq_d, j))
                    nc.sync.dma_start(xT_sb[:, j * T:(j + 1) * T], view_of(xT_d, j))
                # bias gathers are descriptor-heavy; keep them off the
                # critical first trigger slots
                nc.sync.dma_start(bq_sb[:], bqk_d[0].rearrange("(j p) -> p j", p=P))
                nc.sync.dma_start(bk_sb[:], bqk_d[1].rearrange("(j p) -> p j", p=P))
                for j in range(NE):
                    nc.sync.dma_start(wk_sb[:, j * H:(j + 1) * H], view_of(wk_d, j))
                    nc.sync.dma_start(encT_sb[:, j * S:(j + 1) * S], view_of(encT_d, j))
                for j in range(NE):
                    nc.sync.dma_start(wv_sb[:, j * H:(j + 1) * H], view_of(wv_d, j))

                # qT[h-tile i] = sum_j Wq[e_j, h_i]^T @ xT[e_j, :]
                for i in range(NH):
                    ps = psp.tile([P, T], F32, tag="mm")
                    for h0 in range(0, T, 512):
                        for j in range(NE):
                            mm(ps[:, h0:h0 + 512],
                               wq_sb[:, j * H + i * P: j * H + (i + 1) * P],
                               xT_sb[:, j * T + h0: j * T + h0 + 512],
                               start=(j == 0), stop=(j == NE - 1))
                    nc.vector.tensor_scalar_add(
                        qT_sb[:, i * T:(i + 1) * T], ps[:], bq_sb[:, i:i + 1])

                # kT[h-tile i] = sum_j Wk[e_j, h_i]^T @ encT[e_j, :]
                for i in range(NH):
                    ps = psp.tile([P, S], F32, tag="mm")
                    for h0 in range(0, S, 512):
                        for j in range(NE):
                            mm(ps[:, h0:h0 + 512],
                               wk_sb[:, j * H + i * P: j * H + (i + 1) * P],
                               encT_sb[:, j * S + h0: j * S + h0 + 512],
                               start=(j == 0), stop=(j == NE - 1))
                    nc.vector.tensor_scalar_add(
                        kT_sb[:, i * S:(i + 1) * S], ps[:], bk_sb[:, i:i + 1])

                # v[s-tile si] = sum_j encT[e_j, s_si]^T @ Wv[e_j, :]
                for si in range(NS):
                    ps = psp.tile([P, H], F32, tag="mm")
                    for n0, n1 in ((0, 512), (512, H)):
                        for j in range(NE):
                            mm(ps[:, n0:n1],
                               encT_sb[:, j * S + si * P: j * S + (si + 1) * P],
                               wv_sb[:, j * H + n0: j * H + n1],
                               start=(j == 0), stop=(j == NE - 1))
                    nc.scalar.copy(v_sb[:, si * H:(si + 1) * H], ps[:])

            with tc.tile_pool(name="ph2", bufs=1) as ph2:
                wp_sb = ph2.tile([P, NH * E], MM_DT, tag="wp")
                for j in range(NH):
                    nc.sync.dma_start(
                        wp_sb[:, j * E:(j + 1) * E],
                        wp_d[:].rearrange("(j p) e -> j p e", p=P)[j])
                ex_sb = ph2.tile([P, NS * T], MM_DT, tag="ex")
                ot_sb = ph2.tile([P, NH * T], MM_DT, tag="ot")

                # ST[s-tile si] = sum_i kT[h_i, s_si]^T @ qT[h_i, :]; Ex = exp
                for si in range(NS):
                    ps = psp.tile([P, T], F32, tag="mm")
                    for h0 in range(0, T, 512):
                        for i in range(NH):
                            mm(ps[:, h0:h0 + 512],
                               kT_sb[:, i * S + si * P: i * S + (si + 1) * P],
                               qT_sb[:, i * T + h0: i * T + h0 + 512],
                               start=(i == 0), stop=(i == NH - 1))
                    nc.scalar.activation(
                        ex_sb[:, si * T:(si + 1) * T], ps[:], AFT.Exp)

                # r[t] = ones^T @ Ex accumulated over s-tiles -> row [2, T]
                # (ones is the 2-col stationary operand so each of the 16
                # matmuls streams 512 rows instead of paying a 128-col
                # LDWEIGHTS for 1 row of output)
                pr = psr.tile([2, T], F32, tag="r")
                for h0 in range(0, T, 512):
                    for si in range(NS):
                        nc.tensor.matmul(
                            pr[:, h0:h0 + 512],
                            ones_sb[:],
                            ex_sb[:, si * T + h0: si * T + h0 + 512],
                            start=(si == 0), stop=(si == NS - 1))
                nc.vector.reciprocal(rrow_sb[0:1, :], pr[0:1, :])

                # OT[h-tile i] = sum_si v[s_si, h_i]^T @ Ex[s_si, :]
                for i in range(NH):
                    ps = psp.tile([P, T], F32, tag="mm")
                    for h0 in range(0, T, 512):
                        for si in range(NS):
                            mm(ps[:, h0:h0 + 512],
                               v_sb[:, si * H + i * P: si * H + (i + 1) * P],
                               ex_sb[:, si * T + h0: si * T + h0 + 512],
                               start=(si == 0), stop=(si == NS - 1))
                    if i % 2 == 0:
                        nc.scalar.copy(ot_sb[:, i * T:(i + 1) * T], ps[:])
                    else:
                        nc.vector.tensor_copy(ot_sb[:, i * T:(i + 1) * T], ps[:])

                # scatter the reciprocal row [1, T] into per-partition
                # columns [128, NT]: eight K=1 matmuls (row slice as lhsT,
                # scalar 1.0 as rhs) into distinct columns of ONE psum tile
                # so they issue back-to-back, then a single copy out. Placed
                # after the O-phase so the reciprocal latency hides under it.
                pscat = psr.tile([P, NT], F32, tag="r")
                for ti in range(NT):
                    nc.tensor.matmul(
                        pscat[:, ti:ti + 1],
                        rrow_sb[0:1, ti * P:(ti + 1) * P],
                        ones_stg[0:1, 0:1],
                        start=True, stop=True)
                nc.vector.tensor_copy(rcp_sb[:, 0:NT], pscat[:, :])

                # Y[t-tile ti] = (sum_i OT[h_i, t_ti]^T @ Wp[h_i, :]) * rcp[ti]
                for ti in range(NT):
                    ps = psp.tile([P, E], F32, tag="mm")
                    for n0, n1 in ((0, 512), (512, E)):
                        for i in range(NH):
                            mm(ps[:, n0:n1],
                               ot_sb[:, i * T + ti * P: i * T + (ti + 1) * P],
                               wp_sb[:, i * E + n0: i * E + n1],
                               start=(i == 0), stop=(i == NH - 1))
                    y_sb = youtp.tile([P, E], F32, tag="y")
                    nc.vector.tensor_scalar_mul(y_sb[:], ps[:], rcp_sb[:, ti:ti + 1])
                    nc.sync.dma_start(out_d[ti * P:(ti + 1) * P, :], y_sb[:])
    nc.finalize()
    return nc


def get_nc():
    if "nc" not in _NC_CACHE:
        _NC_CACHE["nc"] = _build_bass()
    return _NC_CACHE["nc"]


def kernel(**inputs):
    global LAST_RESULT
    x = np.asarray(inputs["x"], dtype=np.float32)
    enc = np.asarray(inputs["encoder_out"], dtype=np.float32)
    Wq = np.asarray(inputs["Wq"], dtype=np.float32)
    bq = np.asarray(inputs["bq"], dtype=np.float32)
    Wk = np.asarray(inputs["Wk"], dtype=np.float32)
    bk = np.asarray(inputs["bk"], dtype=np.float32)
    Wv = np.asarray(inputs["Wv"], dtype=np.float32)
    bv = np.asarray(inputs["bv"], dtype=np.float32)
    Wp = np.asarray(inputs["Wp"], dtype=np.float32)
    bp = np.asarray(inputs["bp"], dtype=np.float32)

    scale = np.float32(1.0 / np.sqrt(H))
    wq_s = (Wq * scale).astype(np.float32)
    bq_s = (bq * scale).astype(np.float32)
    cvec = (bv @ Wp + bp).astype(np.float32)  # exact rank-1 fold, see header
    bqk = np.ascontiguousarray(np.stack([bq_s, bk]))
    xT = np.ascontiguousarray(x.transpose(0, 2, 1))
    encT = np.ascontiguousarray(enc.transpose(0, 2, 1))

    nc = get_nc()
    in_maps = [
        {"xT": xT[i], "encT": encT[i], "wq": wq_s, "wk": Wk, "wv": Wv,
         "wp": Wp, "bqk": bqk}
        for i in range(B)
    ]
    res = run_bass_kernel_spmd(
        nc, in_maps, list(range(B)),
        trace=bool(os.environ.get("KERNEL_TRACE")),
    )
    LAST_RESULT = res
    out = np.stack([res.results[i]["out"] for i in range(B)])
    if cvec.any():
        out = out + cvec
    return out.astype(np.float32)



# revision 2
# speedup vs baseline: 1.6996x; 1.6996x over previous
"""Single-head cross-attention on 8 NeuronCores, data-parallel over batch.

Math per core (batch element b):
    q = x @ Wq + bq;  k = enc @ Wk + bk;  v = enc @ Wv + bv
    out = softmax(q k^T / sqrt(H)) @ v @ Wp + bp

Weight-fused formulation (exact, host-side folds):
    M  = Wq @ Wk^T / sqrt(H)          [E,E]  host precompute
    Wvp = Wv @ Wp                     [E,E]  host precompute
    scores = x M enc^T (+ row-const from bk: softmax-invariant, dropped;
             + column term from bq: ew[s] = exp(enc[s]@(Wk bq)/sqrt(H)))
    out = (Ex @ [diag(ew)(enc Wvp) | ew]) -> numerator cols 0:768, denom col 768
    bv/bp enter as a host rank-1 add (attn rows sum to 1):  + (bv@Wp + bp)
This drops device work from 4022M MACs to 2816M per core and removes the
separate row-sum/reciprocal-scatter PE phases (denominator rides along as
column 768 of the V operand).

Layout (no on-chip transposes; host pre-transposes x/enc):
    GT[e,t]   = M-tiles as lhsT,   xT as rhs        (= (x@M)^T)
    Vaug[s,:] = encT-tiles as lhsT, Wvp as rhs, scaled by ew[s]; col 768 = ew[s]
    ST[s,t]   = encT-tiles as lhsT, GT as rhs;  Ex = exp(ST)  (no max-sub:
                scores are O(1); softmax shift-invariance keeps it exact)
    O[t,0:769]= Ex-tiles  as lhsT, Vaug as rhs; y = O[:,0:768]/O[:,768]

All matmul operands are bf16 (psum accumulates fp32): same 1 col/cycle PE
rate as fp32r but enables FastWeightLoad so LDWEIGHTS hides under the
streams, and halves DMA bytes. Measured rel err ~3e-3 vs fp32 reference.
"""

import os

import numpy as np
import ml_dtypes

import concourse.bass as bass
import concourse.bacc as bacc
import concourse.tile as tile
from concourse import mybir
from concourse.bass_utils import run_bass_kernel_spmd

P = 128
B, T, S, E, H = 8, 1024, 1024, 768, 768
NE, NT, NS = E // P, T // P, S // P
VA = E + 1  # Vaug row width: 768 value cols + 1 denominator col
F32 = mybir.dt.float32
BF16 = mybir.dt.bfloat16
AFT = mybir.ActivationFunctionType
BF16_NP = ml_dtypes.bfloat16

_NC_CACHE = {}
LAST_RESULT = None


def _build_bass():
    nc = bacc.Bacc()
    xT_d = nc.declare_dram_parameter("xT", [E, T], BF16, isOutput=False)
    encT_d = nc.declare_dram_parameter("encT", [E, S], BF16, isOutput=False)
    mt_d = nc.declare_dram_parameter("mt", [E, E], BF16, isOutput=False)
    wvp_d = nc.declare_dram_parameter("wvp", [E, E], BF16, isOutput=False)
    ew_d = nc.declare_dram_parameter("ew", [P, NS], F32, isOutput=False)
    out_d = nc.declare_dram_parameter("out", [T, E], F32, isOutput=True)

    def mm(ps, lhsT, rhs, start, stop):
        nc.tensor.matmul(ps, lhsT, rhs, start=start, stop=stop)

    with tile.TileContext(nc) as tc:
        with (
            tc.tile_pool(name="const", bufs=1) as constp,
            tc.tile_pool(name="big", bufs=1) as bigp,
            tc.tile_pool(name="psum", bufs=3, space="PSUM") as psp,
            tc.tile_pool(name="yout", bufs=3) as youtp,
        ):
            ew_sb = constp.tile([P, NS], F32, tag="ew")
            mt_sb = bigp.tile([P, NE * E], BF16, tag="mt")
            xt_sb = bigp.tile([P, NE * T], BF16, tag="xt")
            encT_sb = bigp.tile([P, NE * S], BF16, tag="encT")
            wvp_sb = bigp.tile([P, NE * E], BF16, tag="wvp")
            gt_sb = bigp.tile([P, NE * T], BF16, tag="gt")
            ex_sb = bigp.tile([P, NS * T], BF16, tag="ex")
            vaug_sb = bigp.tile([P, NS * VA], BF16, tag="vaug")

            # one trigger per tensor: a single dma_start packet-spreads
            # across all 16 SDMA engines, and each trigger costs ~0.6us of
            # fixed latency on the Sync queue, so fewer+bigger wins
            def load(sb, dram, inner):
                nc.sync.dma_start(
                    sb[:].rearrange("p (j n) -> p j n", j=NE),
                    dram[:].rearrange("(j p) n -> p j n", p=P))

            load(mt_sb, mt_d, E)
            load(xt_sb, xT_d, T)
            load(encT_sb, encT_d, S)
            load(wvp_sb, wvp_d, E)
            nc.sync.dma_start(ew_sb[:], ew_d[:])

            # GT[e-tile i] = sum_j M[e_j, e_i]^T @ xT[e_j, :]
            for i in range(NE):
                ps = psp.tile([P, T], F32, tag="mm")
                for h0 in (0, 512):
                    for j in range(NE):
                        mm(ps[:, h0:h0 + 512],
                           mt_sb[:, j * E + i * P: j * E + (i + 1) * P],
                           xt_sb[:, j * T + h0: j * T + h0 + 512],
                           start=(j == 0), stop=(j == NE - 1))
                if i % 2 == 0:
                    nc.scalar.copy(gt_sb[:, i * T:(i + 1) * T], ps[:])
                else:
                    nc.vector.tensor_copy(gt_sb[:, i * T:(i + 1) * T], ps[:])

            # Vaug[s-tile si] = (sum_j encT[e_j, s_si]^T @ Wvp[e_j, :]) * ew
            # (independent of GT/ST -- placed here so its matmuls cover the
            # GT->SBUF copy latency before ST needs gt_sb)
            for si in range(NS):
                ps = psp.tile([P, E], F32, tag="mm")
                for n0, n1 in ((0, 512), (512, E)):
                    for j in range(NE):
                        mm(ps[:, n0:n1],
                           encT_sb[:, j * S + si * P: j * S + (si + 1) * P],
                           wvp_sb[:, j * E + n0: j * E + n1],
                           start=(j == 0), stop=(j == NE - 1))
                nc.vector.tensor_scalar_mul(
                    vaug_sb[:, si * VA: si * VA + E], ps[:], ew_sb[:, si:si + 1])
                nc.vector.tensor_copy(
                    vaug_sb[:, si * VA + E: (si + 1) * VA], ew_sb[:, si:si + 1])

            # ST[s-tile si] = sum_i encT[e_i, s_si]^T @ GT[e_i, :]; Ex = exp
            for si in range(NS):
                ps = psp.tile([P, T], F32, tag="mm")
                for h0 in (0, 512):
                    for i in range(NE):
                        mm(ps[:, h0:h0 + 512],
                           encT_sb[:, i * S + si * P: i * S + (si + 1) * P],
                           gt_sb[:, i * T + h0: i * T + h0 + 512],
                           start=(i == 0), stop=(i == NE - 1))
                nc.scalar.activation(
                    ex_sb[:, si * T:(si + 1) * T], ps[:], AFT.Exp)

            # O[t-tile ti, 0:769] = sum_si Ex[s_si, t_ti]^T @ Vaug[s_si, :];
            # col 768 is the softmax denominator -- divide and store
            for ti in range(NT):
                ps = psp.tile([P, T], F32, tag="mm")
                for n0, n1 in ((0, 512), (512, VA)):
                    for si in range(NS):
                        mm(ps[:, n0:n1],
                           ex_sb[:, si * T + ti * P: si * T + (ti + 1) * P],
                           vaug_sb[:, si * VA + n0: si * VA + n1],
                           start=(si == 0), stop=(si == NS - 1))
                rc = youtp.tile([P, 1], F32, tag="rcp")
                nc.vector.reciprocal(rc[:], ps[:, E:E + 1])
                y = youtp.tile([P, E], F32, tag="y")
                nc.scalar.activation(y[:], ps[:, 0:E], AFT.Copy, scale=rc[:])
                nc.sync.dma_start(out_d[ti * P:(ti + 1) * P, :], y[:])
    nc.finalize()
    return nc


def get_nc():
    if "nc" not in _NC_CACHE:
        _NC_CACHE["nc"] = _build_bass()
    return _NC_CACHE["nc"]


def kernel(**inputs):
    global LAST_RESULT
    x = np.asarray(inputs["x"], dtype=np.float32)
    enc = np.asarray(inputs["encoder_out"], dtype=np.float32)
    Wq = np.asarray(inputs["Wq"], dtype=np.float32)
    bq = np.asarray(inputs["bq"], dtype=np.float32)
    Wk = np.asarray(inputs["Wk"], dtype=np.float32)
    bk = np.asarray(inputs["bk"], dtype=np.float32)  # noqa: F841  (softmax-invariant)
    Wv = np.asarray(inputs["Wv"], dtype=np.float32)
    bv = np.asarray(inputs["bv"], dtype=np.float32)
    Wp = np.asarray(inputs["Wp"], dtype=np.float32)
    bp = np.asarray(inputs["bp"], dtype=np.float32)

    scale = np.float32(1.0 / np.sqrt(H))
    mt = (Wq @ Wk.T * scale).astype(BF16_NP)           # [E,E]
    wvp = (Wv @ Wp).astype(BF16_NP)                    # [E,E]
    cvec = (bv @ Wp + bp).astype(np.float32)           # exact rank-1 fold
    if bq.any():
        w = (enc @ (Wk @ bq)) * scale                  # [B,S] column term
        ew = np.exp(w, dtype=np.float32)
    else:
        ew = np.ones((B, S), dtype=np.float32)
    ew_in = np.ascontiguousarray(
        ew.reshape(B, NS, P).transpose(0, 2, 1))       # [B,P,NS]
    xT = np.ascontiguousarray(x.transpose(0, 2, 1)).astype(BF16_NP)
    encT = np.ascontiguousarray(enc.transpose(0, 2, 1)).astype(BF16_NP)

    nc = get_nc()
    in_maps = [
        {"xT": xT[i], "encT": encT[i], "mt": mt, "wvp": wvp, "ew": ew_in[i]}
        for i in range(B)
    ]
    res = run_bass_kernel_spmd(
        nc, in_maps, list(range(B)),
        trace=bool(os.environ.get("KERNEL_TRACE")),
    )
    LAST_RESULT = res
    out = np.stack([res.results[i]["out"] for i in range(B)])
    if cvec.any():
        out = out + cvec
    return out.astype(np.float32)
